# revision 21
# baseline (speedup 1.0000x reference)
"""DMoN graph-pooling kernel for 8 Trainium2 NeuronCores.

Math reformulation (no scatter needed):
  S   = softmax(X @ W.T + b)                      [N, k]
  cs  = S.T @ 1                                   [k]      (cluster_sizes)
  M   = S.T @ X                                   [k, D]
  T   = sum_e w_e * S[c_e] (x) S[r_e]             [k, k]   (= (A@S).T @ S reordered)
  v   = sum_e w_e * S[c_e]                        [k]      (= S.T @ degrees)
  E   = sum_e w_e
  trace(graph_pooled)  = trace(T)
  trace(normalizer)    = (v . v) / (2E)
  spectral_loss        = -(trace(T) - (v.v)/(2E)) / (2E)
  collapse_loss        = 0.1 * (||cs|| / n * sqrt(k) - 1)
  features_pooled      = selu(M / cs[:, None])

Distribution: all 8 cores replicate the softmax pass (each needs the full S
in its own HBM for edge gathers), but each core only accumulates cs/M and
writes the f32 S output for its own 1/8 node slice, and processes its own
1/8 edge shard.  SPMD divergence is avoided by rotating the node array per
core (host-side np.roll) so that "my slice" is always rows [0, SLICE) —
edge endpoint indices are remapped to the rotated coordinates per core.
Final tiny combines (sums of [64,65]-sized partials, selu, losses) happen
on the host.
"""

import numpy as np


def _ensure_paths():
    import sys
    try:
        import concourse  # noqa: F401
        return
    except ImportError:
        pass
    for p in ("/opt/trn_rl_repo", "/root/.axon_site/_ro/trn_rl_repo"):
        if p not in sys.path:
            sys.path.insert(0, p)
    import concourse  # noqa: F401


# ---------------------------------------------------------------- config ---

BLK = 32768          # node rows per gather block (int16 index range)
GTILE = 8192         # max edges per dma_gather call (16384 overflows the
                     # SWDGE descriptor ring and wedges the device)


class CFG:
    """Full-size problem configuration."""
    N_NODES = 100000
    D = 128
    K = 64
    N_EDGES = 3200000
    N_CORES = 8

    NB = 7                # node chunks (of 128 rows) per phase-1 batch
    SLICE_BATCHES = 14    # batches per core slice

    @classmethod
    def derived(cls):
        cls.SLICE_CHUNKS = cls.NB * cls.SLICE_BATCHES          # 98
        cls.SLICE = cls.SLICE_CHUNKS * 128                     # 12544
        cls.NPAD = cls.SLICE * cls.N_CORES                     # 100352
        cls.NBATCH = cls.NPAD // (cls.NB * 128)                # 112
        cls.EPC = cls.N_EDGES // cls.N_CORES                   # 400000
        assert cls.NPAD >= cls.N_NODES

        # edge buckets: (col_block, row_block) pairs with static capacities
        cls.NBLK = (cls.NPAD + BLK - 1) // BLK
        widths = [min(BLK, cls.NPAD - BLK * b) for b in range(cls.NBLK)]
        probs = [min(w, cls.N_NODES) / cls.N_NODES for w in widths]
        caps = []
        for bc in range(cls.NBLK):
            for br in range(cls.NBLK):
                p = probs[bc] * probs[br]
                mu = cls.EPC * p
                sig = (cls.EPC * p * (1 - p)) ** 0.5
                cap = int(mu + 6.5 * sig + 64)
                cap = ((cap + 2047) // 2048) * 2048
                caps.append((bc, br, cap))
        cls.BUCKETS = caps
        cls.EPAD = sum(c for _, _, c in caps)
        return cls


CFG.derived()


def _split_excess_waits(nc, max_waits=1):
    """walrus's CoreV3 codegen rejects instructions carrying more than ~2
    semaphore waits ("Too many sync wait commands").  Move excess waits onto
    dedicated same-engine NoOps inserted immediately before the instruction
    (engine FIFO order preserves the semantics)."""
    from concourse import mybir

    ctr = 0
    for fn in nc.m.functions:
        for bb in fn.blocks:
            new = []
            for ins in bb.instructions:
                si = ins.sync_info
                waits = list(si.on_wait) if si is not None and (si.on_wait or []) else []
                if len(waits) > max_waits:
                    extra, keep = waits[:-max_waits], waits[-max_waits:]
                    for w in extra:
                        n = mybir.InstNoOp(name=f"I-wsplit-{ctr}", ins=[], outs=[])
                        ctr += 1
                        n.engine = ins.engine
                        n.sync_info = mybir.SyncInfo(on_wait=[w], on_update=[])
                        new.append(n)
                    ins.sync_info = mybir.SyncInfo(
                        on_wait=keep, on_update=list(si.on_update or []))
                new.append(ins)
            bb.instructions = new


# ---------------------------------------------------------------- kernel ---

def build_nc(cfg=CFG):
    _ensure_paths()
    import concourse.bass as bass
    import concourse.tile as tile
    from concourse import mybir
    from concourse.masks import make_identity

    f32 = mybir.dt.float32
    f16 = mybir.dt.float16
    i32 = mybir.dt.int32
    AX = mybir.AxisListType
    OP = mybir.AluOpType
    AF = mybir.ActivationFunctionType

    from concourse import library_config
    from concourse.library_overlay import lower_extended_insts

    NB, K, D = cfg.NB, cfg.K, cfg.D
    NPAD, NBATCH, SLICE_BATCHES = cfg.NPAD, cfg.NBATCH, cfg.SLICE_BATCHES
    SLICE, SLICE_CHUNKS = cfg.SLICE, cfg.SLICE_CHUNKS
    EPAD = cfg.EPAD
    RP = 128   # padded fp16 row length of the gather copy of S (256 bytes)
    i16 = mybir.dt.int16

    nc = bass.Bass()

    x_in = nc.dram_tensor("features", [NPAD, D], f32, kind="ExternalInput")
    wt_in = nc.dram_tensor("w_t", [D, K], f32, kind="ExternalInput")
    mask_in = nc.dram_tensor("mask", [128, SLICE_CHUNKS], f32, kind="ExternalInput")
    idxr_in = nc.dram_tensor("idxr", [32, EPAD // 16], i16, kind="ExternalInput")
    idxc_in = nc.dram_tensor("idxc", [32, EPAD // 16], i16, kind="ExternalInput")
    w_in = nc.dram_tensor("wmat", [128, EPAD // 128], f32, kind="ExternalInput")

    s_out = nc.dram_tensor("s_out", [SLICE, K], f32, kind="ExternalOutput")
    m_out = nc.dram_tensor("m_out", [K, D], f32, kind="ExternalOutput")
    cs_out = nc.dram_tensor("cs_out", [K, 1], f32, kind="ExternalOutput")
    t_out = nc.dram_tensor("t_out", [K, K + 1], f32, kind="ExternalOutput")

    # fp16 copy of S, rows padded to 256B for dma_gather.  Only cols [0,64)
    # are written; col 64 of gathered tiles is overwritten with ones (for the
    # v / degree accumulation), cols 65.. are never read.
    s16 = nc.dram_tensor("s16", [NPAD, RP], f16)

    # batched DRAM views: batch t, partition p, chunk j, feature d
    x_v = x_in[:].rearrange("(t j p) d -> t p j d", p=128, j=NB)
    s16_v = s16[:, 0:K].rearrange("(t j p) k -> t p j k", p=128, j=NB)
    sout_v = s_out[:].rearrange("(t j p) k -> t p j k", p=128, j=NB)

    nc.gpsimd.load_library(library_config.mlp)

    with tile.TileContext(nc) as tc:
        with (
            tc.tile_pool(name="const", bufs=1) as cpool,
            tc.tile_pool(name="ph1", bufs=2) as pool1,
            tc.tile_pool(name="ph1ps", bufs=2, space="PSUM") as ppool1,
            tc.tile_pool(name="accps", bufs=1, space="PSUM") as apool,
        ):
            identity = cpool.tile([128, 128], f32)
            make_identity(nc, identity[:])
            wt_sb = cpool.tile([D, K], f32)
            nc.sync.dma_start(out=wt_sb[:], in_=wt_in[:])
            mask_sb = cpool.tile([128, SLICE_CHUNKS], f32)
            nc.sync.dma_start(out=mask_sb[:], in_=mask_in[:])

            m_ps = apool.tile([K, D], f32, space="PSUM")
            cs_ps = apool.tile([K, 1], f32, space="PSUM")

            # ---------------- phase 1: softmax assignments -----------------
            for t in range(NBATCH):
                in_slice = t < SLICE_BATCHES
                xb = pool1.tile([128, NB, D], f32, tag="xb")
                nc.sync.dma_start(out=xb[:], in_=x_v[t])

                xt_ps = ppool1.tile([128, NB, 128], f32, space="PSUM", tag="xt")
                for j in range(NB):
                    nc.tensor.transpose(xt_ps[:, j, :], xb[:, j, :], identity[:])
                xt_sb = pool1.tile([128, NB, 128], f32, tag="xt_sb")
                nc.vector.tensor_copy(xt_sb[:], xt_ps[:])

                lg_ps = ppool1.tile([128, NB, K], f32, space="PSUM", tag="lg")
                for j in range(NB):
                    nc.tensor.matmul(lg_ps[:, j, :], xt_sb[:, j, :], wt_sb[:],
                                     start=True, stop=True)

                ex = pool1.tile([128, NB, K], f32, tag="ex")
                nc.scalar.activation(ex[:], lg_ps[:], AF.Exp)
                sums = pool1.tile([128, NB], f32, tag="sums")
                nc.vector.tensor_reduce(sums[:], ex[:], axis=AX.X, op=OP.add)
                rec = pool1.tile([128, NB], f32, tag="rec")
                nc.vector.reciprocal(rec[:], sums[:])

                s16b = pool1.tile([128, NB, K], f16, tag="s16b")
                if in_slice:
                    s32b = pool1.tile([128, NB, K], f32, tag="s32b")
                    nc.vector.tensor_tensor(
                        out=s32b[:], in0=ex[:],
                        in1=rec[:].to_broadcast([128, NB, K]), op=OP.mult)
                    nc.vector.tensor_copy(s16b[:], s32b[:])
                    nc.sync.dma_start(out=sout_v[t], in_=s32b[:])
                    for j in range(NB):
                        c = t * NB + j
                        nc.tensor.matmul(m_ps[:], s32b[:, j, :], xb[:, j, :],
                                         start=(c == 0), stop=(c == SLICE_CHUNKS - 1))
                        nc.tensor.matmul(cs_ps[:], s32b[:, j, :],
                                         mask_sb[:, c:c + 1],
                                         start=(c == 0), stop=(c == SLICE_CHUNKS - 1))
                else:
                    nc.vector.tensor_tensor(
                        out=s16b[:], in0=ex[:],
                        in1=rec[:].to_broadcast([128, NB, K]), op=OP.mult)
                nc.sync.dma_start(out=s16_v[t], in_=s16b[:])

            m_sb = pool1.tile([K, D], f32, tag="m_sb")
            nc.vector.tensor_copy(m_sb[:], m_ps[:])
            nc.sync.dma_start(out=m_out[:], in_=m_sb[:])
            cs_sb = pool1.tile([K, 1], f32, tag="cs_sb")
            nc.vector.tensor_copy(cs_sb[:], cs_ps[:])
            nc.sync.dma_start(out=cs_out[:], in_=cs_sb[:])

        # ------------------- phase 2: edge contraction ---------------------
        # T_ext = sum_e w_e * S[c_e] (x) [S[r_e] | 1]  accumulated in PSUM.
        # Edges are pre-bucketed by (col_block, row_block) so the dma_gather
        # int16 indices are block-local; each bucket has a static capacity.
        ncols_total = EPAD // 128
        with (
            tc.tile_pool(name="ph2", bufs=2) as pool2,
            tc.tile_pool(name="ph2ps", bufs=1, space="PSUM") as ppool2,
        ):
            t_ps = ppool2.tile([K, K + 1], f32, space="PSUM")
            # one shared register per distinct gather size (to_reg allocates
            # a fresh register per call and the pool is small)
            sizes = set()
            for (_, _, cap) in cfg.BUCKETS:
                rem = cap
                while rem > 0:
                    sizes.add(min(rem, GTILE))
                    rem -= min(rem, GTILE)
            nreg = {s: nc.gpsimd.to_reg(s) for s in sorted(sizes)}
            off = 0
            col = 0
            for (bc, br, cap) in cfg.BUCKETS:
                src_c = s16[bc * BLK:min((bc + 1) * BLK, NPAD), :]
                src_r = s16[br * BLK:min((br + 1) * BLK, NPAD), :]
                rem = cap
                while rem > 0:
                    n = min(rem, GTILE)
                    G = n // 128
                    idc = pool2.tile([128, GTILE // 16], i16, tag="idc")
                    nc.sync.dma_start(
                        out=idc[0:32, 0:n // 16],
                        in_=idxc_in[:, off // 16:(off + n) // 16])
                    idr = pool2.tile([128, GTILE // 16], i16, tag="idr")
                    nc.sync.dma_start(
                        out=idr[0:32, 0:n // 16],
                        in_=idxr_in[:, off // 16:(off + n) // 16])
                    wv = pool2.tile([128, GTILE // 128], f32, tag="wv")
                    nc.sync.dma_start(
                        out=wv[:, 0:G],
                        in_=w_in[:, off // 128:(off + n) // 128])
                    wv16 = pool2.tile([128, GTILE // 128], f16, tag="wv16")
                    nc.vector.tensor_copy(wv16[:, 0:G], wv[:, 0:G])

                    gc = pool2.tile([128, GTILE // 128, RP], f16, tag="gc")
                    nc.gpsimd.dma_gather(
                        out_ap=gc[:, 0:G, :], in_ap=src_c,
                        idxs_ap=idc[:, 0:n // 16],
                        num_idxs=n, num_idxs_reg=nreg[n], elem_size=RP,
                        single_packet=False)
                    gr = pool2.tile([128, GTILE // 128, RP], f16, tag="gr")
                    nc.gpsimd.dma_gather(
                        out_ap=gr[:, 0:G, :], in_ap=src_r,
                        idxs_ap=idr[:, 0:n // 16],
                        num_idxs=n, num_idxs_reg=nreg[n], elem_size=RP,
                        single_packet=False)
                    # ones column for the v / E accumulation
                    nc.vector.memset(gr[:, 0:G, K:K + 1], 1.0)

                    sc = pool2.tile([128, GTILE // 128, K], f16, tag="sc")
                    nc.vector.tensor_tensor(
                        out=sc[:, 0:G, :], in0=gc[:, 0:G, 0:K],
                        in1=wv16[:, 0:G].to_broadcast([128, G, K]),
                        op=OP.mult)

                    for g in range(G):
                        nc.tensor.matmul(t_ps[:], sc[:, g, :], gr[:, g, 0:K + 1],
                                         start=(col == 0),
                                         stop=(col == ncols_total - 1))
                        col += 1
                    off += n
                    rem -= n

            t_sb = pool2.tile([K, K + 1], f32, tag="t_sb")
            nc.vector.tensor_copy(t_sb[:], t_ps[:])
            nc.sync.dma_start(out=t_out[:], in_=t_sb[:])

    lower_extended_insts(nc)
    _split_excess_waits(nc, max_waits=1)
    return nc


_NC_CACHE = {}


def _get_nc(cfg=CFG):
    key = (cfg.NPAD, cfg.EPAD, cfg.NB)
    if key not in _NC_CACHE:
        _NC_CACHE[key] = build_nc(cfg)
    return _NC_CACHE[key]


def make_in_maps(features, W, b, edge_vals, edge_rows, edge_cols, cfg=CFG):
    n, d = features.shape
    k = W.shape[0]
    assert (n, d, k) == (cfg.N_NODES, cfg.D, cfg.K)

    x_pad = np.zeros((cfg.NPAD, cfg.D), dtype=np.float32)
    x_pad[:n] = np.asarray(features, dtype=np.float32)
    w_t = np.ascontiguousarray(np.asarray(W, dtype=np.float32).T)

    ev = np.asarray(edge_vals, dtype=np.float32)
    er = np.asarray(edge_rows, dtype=np.int64)
    ec = np.asarray(edge_cols, dtype=np.int64)

    in_maps = []
    for core in range(cfg.N_CORES):
        shift = cfg.SLICE * core
        x_rot = np.roll(x_pad, -shift, axis=0) if shift else x_pad.copy()

        # per-core slice validity mask [p, c] for global row shift + c*128 + p
        p = np.arange(128)[:, None]
        c = np.arange(cfg.SLICE_CHUNKS)[None, :]
        gl = shift + c * 128 + p
        mask = (gl < n).astype(np.float32)

        lo, hi = core * cfg.EPC, (core + 1) * cfg.EPC
        er_k = (er[lo:hi] - shift) % cfg.NPAD
        ec_k = (ec[lo:hi] - shift) % cfg.NPAD
        ew_k = ev[lo:hi]

        # bucket edges by (col_block, row_block); block-local int16 indices
        bc = ec_k // BLK
        br = er_k // BLK
        bucket = bc * cfg.NBLK + br
        order = np.argsort(bucket, kind="stable")
        bkt_sorted = bucket[order]
        ec_s = (ec_k - bc * BLK)[order].astype(np.int16)
        er_s = (er_k - br * BLK)[order].astype(np.int16)
        ew_s = ew_k[order]

        ec_stream = np.zeros(cfg.EPAD, dtype=np.int16)
        er_stream = np.zeros(cfg.EPAD, dtype=np.int16)
        ew_stream = np.zeros(cfg.EPAD, dtype=np.float32)
        starts = np.searchsorted(bkt_sorted, np.arange(cfg.NBLK * cfg.NBLK))
        ends = np.searchsorted(bkt_sorted, np.arange(cfg.NBLK * cfg.NBLK) + 1)
        off = 0
        for i, (bcb, brb, cap) in enumerate(cfg.BUCKETS):
            s, e = starts[i], ends[i]
            cnt = e - s
            assert cnt <= cap, f"bucket {i} overflow: {cnt} > {cap}"
            ec_stream[off:off + cnt] = ec_s[s:e]
            er_stream[off:off + cnt] = er_s[s:e]
            ew_stream[off:off + cnt] = ew_s[s:e]
            off += cap

        # device layouts: idx wrapped [16, n/16] duplicated into 32 rows
        # (the gather ucode's tx core reads partitions 16..31, rx 0..15);
        # w as [128, n/128] with edge i at [i%128, i//128].
        idxc16 = ec_stream.reshape(-1, 16).T
        idxr16 = er_stream.reshape(-1, 16).T
        idxc32 = np.ascontiguousarray(np.concatenate([idxc16, idxc16], axis=0))
        idxr32 = np.ascontiguousarray(np.concatenate([idxr16, idxr16], axis=0))
        wmat = np.ascontiguousarray(ew_stream.reshape(-1, 128).T)

        in_maps.append({
            "features": x_rot,
            "w_t": w_t,
            "mask": np.ascontiguousarray(mask),
            "idxr": idxr32,
            "idxc": idxc32,
            "wmat": wmat,
        })
    return in_maps


def combine_outputs(results, edge_vals, cfg=CFG):
    """results: list of per-core output dicts."""
    n, k = cfg.N_NODES, cfg.K
    s_pad = np.concatenate([results[c]["s_out"] for c in range(cfg.N_CORES)], axis=0)
    assignments = np.ascontiguousarray(s_pad[:n]).astype(np.float32)

    cs = np.sum([results[c]["cs_out"][:, 0] for c in range(cfg.N_CORES)],
                axis=0, dtype=np.float64)
    m = np.sum([results[c]["m_out"] for c in range(cfg.N_CORES)],
               axis=0, dtype=np.float64)
    t_ext = np.sum([results[c]["t_out"] for c in range(cfg.N_CORES)],
                   axis=0, dtype=np.float64)
    t_mat = t_ext[:, :k]
    v = t_ext[:, k]

    e_tot = float(np.sum(np.asarray(edge_vals, dtype=np.float64)))
    t1 = float(np.trace(t_mat))
    vv = float(np.dot(v, v))
    spectral_loss = np.float32(-(t1 - vv / (2.0 * e_tot)) / (2.0 * e_tot))

    collapse_loss = np.float32(
        0.1 * (np.linalg.norm(cs) / n * np.sqrt(k) - 1.0))

    # features_pooled = selu(M / cs[:, None])  (jax.nn.selu constants)
    scale = 1.0507009873554805
    alpha = 1.6732632423543772
    x = (m / cs[:, None]).astype(np.float32)
    features_pooled = np.where(
        x > 0, scale * x, np.float32(scale * alpha) * np.expm1(x)
    ).astype(np.float32)

    return features_pooled, assignments, spectral_loss, collapse_loss


_last_results = None


def _numpy_fallback(features, W, b, edge_vals, edge_rows, edge_cols):
    """Exact numpy implementation; only used if b != 0 (never in practice —
    the device kernel folds softmax without the always-zero bias)."""
    n, d = features.shape
    k = W.shape[0]
    logits = features @ W.T + b
    e = np.exp(logits - logits.max(axis=1, keepdims=True))
    S = (e / e.sum(axis=1, keepdims=True)).astype(np.float32)
    cs = S.sum(axis=0, dtype=np.float64)
    deg = np.zeros(n); np.add.at(deg, edge_cols, edge_vals.astype(np.float64))
    AS = np.zeros((n, k))
    np.add.at(AS, edge_rows, edge_vals[:, None].astype(np.float64) * S[edge_cols])
    gp = AS.T @ S
    e2 = 2 * deg.sum()
    nl = S.T.astype(np.float64) @ deg
    spectral = np.float32(-(np.trace(gp) - np.dot(nl, nl) / e2) / e2)
    collapse = np.float32(0.1 * (np.linalg.norm(cs) / n * np.sqrt(k) - 1.0))
    M = S.T.astype(np.float64) @ features
    x = (M / cs[:, None]).astype(np.float32)
    scale, alpha = 1.0507009873554805, 1.6732632423543772
    fp = np.where(x > 0, scale * x,
                  np.float32(scale * alpha) * np.expm1(x)).astype(np.float32)
    return fp, S, spectral, collapse


def kernel(features, W, b, edge_vals, edge_rows, edge_cols):
    global _last_results
    if np.any(np.asarray(b) != 0):
        return _numpy_fallback(np.asarray(features), np.asarray(W),
                               np.asarray(b), np.asarray(edge_vals),
                               np.asarray(edge_rows), np.asarray(edge_cols))
    _ensure_paths()
    from concourse.bass_utils import run_bass_kernel_spmd

    cfg = CFG
    nc = _get_nc(cfg)
    in_maps = make_in_maps(features, W, b, edge_vals, edge_rows, edge_cols, cfg)
    res = run_bass_kernel_spmd(nc, in_maps, core_ids=list(range(cfg.N_CORES)))
    _last_results = res
    return combine_outputs(res.results, edge_vals, cfg)
